# revision 18
# baseline (speedup 1.0000x reference)
"""Distributed Trainium2 Bass kernel for nn_Attention_1726576855421.

Dense GQA attention block (dim 4096, 32 q-heads / 8 kv-heads, head_dim 128,
seq 2048, start_pos 0) tensor-parallel over heads across 8 NeuronCores:
  - core c owns q-heads [4c, 4c+4) and kv-head c
  - wq/wk/wv sharded on the output dim, wo sharded on the OUTPUT dim
    (each core computes a 512-wide column slice of the final output, so the
    host just concatenates along the feature axis; the only collective is a
    chunked AllGather of the per-core attention outputs).

Compute dtype: bf16 operands with fp32 PSUM accumulation (rel-err ~1e-3).
Scores are computed transposed (ST[k, q]) so that:
  - softmax denominator = ones-vector matmul over partitions,
  - P@V needs no transposition of P (lhsT = V natural, rhs = exp(ST)).
"""

import sys

for _p in ("/opt/trn_rl_repo", "/root/.axon_site/_ro/trn_rl_repo"):
    if _p not in sys.path:
        sys.path.append(_p)

import numpy as np

# problem constants (hardcoded per the task statement)
S = 2048          # sequence length
D = 4096          # model dim
NCORES = 8
H = 4             # q heads per core
DH = 128          # head dim
P = 128           # partitions
OQ = H * DH       # 512, per-core q-projection width
NDT = D // P      # 32 d-tiles
NKT = S // P      # 16 k-tiles
SC = 512          # s-chunk (free dim of most matmuls)
NCH = S // SC     # 4 chunks
NEG_CLAMP = -60.0
INV_SQRT_DH = float(1.0 / np.sqrt(DH))

MODE_NONE = "none"       # mask is all zeros -> no masking at all
MODE_CAUSAL = "causal"   # mask == triu(NEG_INF, k=1) -> skip masked tiles
MODE_GENERAL = "general" # arbitrary additive mask

_BUILD_CACHE = {}


def _build(mask_mode):
    import ml_dtypes
    import concourse.bacc as bacc
    import concourse.bass as bass
    import concourse.tile as tile
    import concourse.mybir as mybir

    f32 = mybir.dt.float32
    f32r = mybir.dt.float32r
    bf16 = mybir.dt.bfloat16
    EXP = mybir.ActivationFunctionType.Exp
    COPY = mybir.ActivationFunctionType.Copy
    MULT = mybir.AluOpType.mult
    ADD = mybir.AluOpType.add
    MAXOP = mybir.AluOpType.max
    npbf = ml_dtypes.bfloat16

    nc = bacc.Bacc(None, target_bir_lowering=False, debug=False)

    x_p = nc.declare_dram_parameter("x", [S, D], f32, isOutput=False)
    wq_p = nc.declare_dram_parameter("wq", [OQ, D], f32, isOutput=False)
    wk_p = nc.declare_dram_parameter("wk", [DH, D], f32, isOutput=False)
    wv_p = nc.declare_dram_parameter("wv", [DH, D], f32, isOutput=False)
    wo_p = nc.declare_dram_parameter("wo", [OQ, D], f32, isOutput=False)
    cos_p = nc.declare_dram_parameter("cosf", [S, DH // 2], f32, isOutput=False)
    sin_p = nc.declare_dram_parameter("sinf", [S, DH // 2], f32, isOutput=False)
    if mask_mode != MODE_NONE:
        mask_p = nc.declare_dram_parameter("mask", [S, S], f32, isOutput=False)
    out_p = nc.declare_dram_parameter("out", [S, OQ], f32, isOutput=True)

    # constants baked into the NEFF
    eye_bf_d = nc.inline_tensor(np.eye(P, dtype=npbf), name="eye_bf")
    eye_f_d = nc.inline_tensor(np.eye(P, dtype=np.float32), name="eye_f")
    rswap = np.zeros((P, P), npbf)
    for i in range(P):
        rswap[i ^ 1, i] = 1.0
    rswap_d = nc.inline_tensor(rswap, name="rswap")
    ones_col_d = nc.inline_tensor(np.ones((P, 1), dtype=npbf), name="ones_col")

    # which k-tiles are live / need the additive mask, per q-chunk
    def k_tiles_for(qc):
        if mask_mode == MODE_NONE:
            return list(range(NKT)), set()
        if mask_mode == MODE_GENERAL:
            return list(range(NKT)), set(range(NKT))
        # causal: k-tile fully unmasked iff kt*128+127 <= qc*512 (min q)
        live = list(range(4 * qc + 4))
        diag = set(range(4 * qc, 4 * qc + 4))
        return live, diag

    with tile.TileContext(nc) as tc:
        from contextlib import ExitStack

        with ExitStack() as top:
            consts = top.enter_context(tc.tile_pool(name="consts", bufs=1))
            dram = top.enter_context(tc.tile_pool(name="dram", bufs=1, space="DRAM"))

            eye_bf = consts.tile([P, P], bf16)
            nc.sync.dma_start(out=eye_bf, in_=eye_bf_d[:, :])
            eye_f = consts.tile([P, P], f32)
            nc.sync.dma_start(out=eye_f, in_=eye_f_d[:, :])
            rsw = consts.tile([P, P], bf16)
            nc.sync.dma_start(out=rsw, in_=rswap_d[:, :])
            ones_col = consts.tile([P, 1], bf16)
            nc.sync.dma_start(out=ones_col, in_=ones_col_d[:, :])

            # persistent activations
            qt = consts.tile([P, H, S], bf16)       # 2 MB, rope'd Q^T per head
            kt = consts.tile([P, S], bf16)          # 0.5 MB, rope'd K^T
            v_sb = consts.tile([P, NKT, DH], bf16)  # 0.5 MB, V natural

            # ---------------- phase 0c+1: weights + QKV ----------------
            with ExitStack() as p1:
                rope_consts = p1.enter_context(
                    tc.tile_pool(name="rope_consts", bufs=1))
                ct = rope_consts.tile([P, S], bf16)    # cos multiplier (transposed)
                st_m = rope_consts.tile([P, S], bf16)  # +-sin multiplier (transposed)

                # rope multiplier prep
                with tc.tile_pool(name="rope_prep", bufs=2) as rp, \
                     tc.tile_pool(name="rp_ps", bufs=2, space="PSUM") as rp_ps:
                    cos_sb = rp.tile([P, NKT, DH // 2], f32, tag="cs")
                    nc.sync.dma_start(
                        out=cos_sb,
                        in_=cos_p.ap().rearrange("(t p) f -> p t f", p=P)
                    )
                    sin_sb = rp.tile([P, NKT, DH // 2], f32, tag="cs")
                    nc.sync.dma_start(
                        out=sin_sb,
                        in_=sin_p.ap().rearrange("(t p) f -> p t f", p=P)
                    )
                    cexp = rp.tile([P, NKT, DH], bf16, tag="ce")
                    sexp = rp.tile([P, NKT, DH], bf16, tag="ce")
                    cview = cexp.rearrange("p t (f two) -> p t f two", two=2)
                    sview = sexp.rearrange("p t (f two) -> p t f two", two=2)
                    nc.vector.tensor_copy(cview[:, :, :, 0], cos_sb)
                    nc.vector.tensor_copy(cview[:, :, :, 1], cos_sb)
                    # S'[s, 2i] = -sin[s, i], S'[s, 2i+1] = +sin[s, i]
                    nc.vector.tensor_scalar_mul(sview[:, :, :, 0], sin_sb, -1.0)
                    nc.vector.tensor_copy(sview[:, :, :, 1], sin_sb)
                    for t in range(NKT):
                        cps = rp_ps.tile([P, P], bf16, tag="cps")
                        nc.tensor.transpose(cps, cexp[:, t, :], eye_bf)
                        nc.vector.tensor_copy(ct[:, t * P:(t + 1) * P], cps)
                        sps = rp_ps.tile([P, P], bf16, tag="cps")
                        nc.tensor.transpose(sps, sexp[:, t, :], eye_bf)
                        nc.vector.tensor_copy(st_m[:, t * P:(t + 1) * P], sps)

                wst = p1.enter_context(tc.tile_pool(name="wstage", bufs=2))
                wpool = p1.enter_context(tc.tile_pool(name="wqkvT", bufs=1))
                xst = p1.enter_context(tc.tile_pool(name="xstage", bufs=2))
                xtp = p1.enter_context(tc.tile_pool(name="xt", bufs=1))
                vtp = p1.enter_context(tc.tile_pool(name="vt", bufs=1))
                ropep = p1.enter_context(tc.tile_pool(name="ropep", bufs=2))
                tp_ps = p1.enter_context(
                    tc.tile_pool(name="tp_ps", bufs=2, space="PSUM"))
                qkv_ps = p1.enter_context(
                    tc.tile_pool(name="qkv_ps", bufs=2, space="PSUM"))
                rot_ps = p1.enter_context(
                    tc.tile_pool(name="rot_ps", bufs=2, space="PSUM"))

                wqT = wpool.tile([P, NDT, OQ], bf16)
                wkT = wpool.tile([P, NDT, DH], bf16)
                wvT = wpool.tile([P, NDT, DH], bf16)
                vt_sb = vtp.tile([P, S], bf16)

                def transpose_weight(w_param, n_pt, wT, evac_engines):
                    # w [n_pt*128, 4096] f32 DRAM -> wT [128, 32, n_pt*128] bf16
                    for pt in range(n_pt):
                        stg = wst.tile([P, D], bf16, tag="wstg")
                        nc.gpsimd.dma_start(
                            out=stg, in_=w_param[pt * P:(pt + 1) * P, :])
                        for dg in range(NDT // 4):
                            ps = tp_ps.tile([P, 4, P], bf16, tag="tps")
                            for j in range(4):
                                dt_i = dg * 4 + j
                                nc.tensor.transpose(
                                    ps[:, j, :],
                                    stg[:, dt_i * P:(dt_i + 1) * P], eye_bf)
                            eng = evac_engines[dg % len(evac_engines)]
                            eng.tensor_copy(
                                wT[:, dg * 4:dg * 4 + 4, pt * P:(pt + 1) * P], ps)

                transpose_weight(wq_p, 4, wqT, [nc.vector])
                transpose_weight(wk_p, 1, wkT, [nc.vector])
                transpose_weight(wv_p, 1, wvT, [nc.vector])

                def rope_evac(psum, rot_src, dst, c):
                    # dst[:, c*SC:(c+1)*SC] = psum*ct + (R@psum)*st  (all rope'd)
                    raw = ropep.tile([P, SC], bf16, tag="raw")
                    nc.scalar.activation(raw, psum, COPY)
                    rps = rot_ps.tile([P, SC], f32, tag="rot")
                    nc.tensor.matmul(rps, rsw, raw, start=True, stop=True)
                    rotb = ropep.tile([P, SC], bf16, tag="rotb")
                    nc.scalar.activation(rotb, rps, COPY)
                    t1 = ropep.tile([P, SC], bf16, tag="t1")
                    nc.vector.tensor_tensor(
                        t1, raw, ct[:, c * SC:(c + 1) * SC], MULT)
                    t2 = ropep.tile([P, SC], bf16, tag="t2")
                    nc.vector.tensor_tensor(
                        t2, rotb, st_m[:, c * SC:(c + 1) * SC], MULT)
                    nc.vector.tensor_tensor(dst, t1, t2, ADD)

                for c in range(NCH):
                    xt_c = xtp.tile([P, NDT, SC], bf16, tag="xt")
                    for ss in range(4):
                        s_tile = c * 4 + ss
                        stg = xst.tile([P, D], bf16, tag="xstg")
                        nc.gpsimd.dma_start(
                            out=stg, in_=x_p[s_tile * P:(s_tile + 1) * P, :])
                        for dg in range(NDT // 4):
                            ps = tp_ps.tile([P, 4, P], bf16, tag="tps")
                            for j in range(4):
                                dt_i = dg * 4 + j
                                nc.tensor.transpose(
                                    ps[:, j, :],
                                    stg[:, dt_i * P:(dt_i + 1) * P], eye_bf)
                            eng = nc.vector if (dg % 2 == 0) else nc.scalar
                            if dg % 2 == 0:
                                nc.vector.tensor_copy(
                                    xt_c[:, dg * 4:dg * 4 + 4,
                                         ss * P:(ss + 1) * P], ps)
                            else:
                                nc.scalar.activation(
                                    xt_c[:, dg * 4:dg * 4 + 4,
                                         ss * P:(ss + 1) * P], ps, COPY)
                    # Q^T per head
                    for h in range(H):
                        ps = qkv_ps.tile([P, SC], f32, tag="qkv")
                        for d in range(NDT):
                            nc.tensor.matmul(
                                ps, wqT[:, d, h * P:(h + 1) * P], xt_c[:, d, :],
                                start=(d == 0), stop=(d == NDT - 1))
                        rope_evac(ps, rot_ps, qt[:, h, c * SC:(c + 1) * SC], c)
                    # K^T
                    ps = qkv_ps.tile([P, SC], f32, tag="qkv")
                    for d in range(NDT):
                        nc.tensor.matmul(
                            ps, wkT[:, d, :], xt_c[:, d, :],
                            start=(d == 0), stop=(d == NDT - 1))
                    rope_evac(ps, rot_ps, kt[:, c * SC:(c + 1) * SC], c)
                    # V^T (no rope)
                    ps = qkv_ps.tile([P, SC], f32, tag="qkv")
                    for d in range(NDT):
                        nc.tensor.matmul(
                            ps, wvT[:, d, :], xt_c[:, d, :],
                            start=(d == 0), stop=(d == NDT - 1))
                    nc.scalar.activation(
                        vt_sb[:, c * SC:(c + 1) * SC], ps, COPY)

                # V natural [s, d] from V^T
                for tg in range(NKT // 4):
                    ps = tp_ps.tile([P, 4, P], bf16, tag="tps")
                    for j in range(4):
                        t = tg * 4 + j
                        nc.tensor.transpose(
                            ps[:, j, :], vt_sb[:, t * P:(t + 1) * P], eye_bf)
                    nc.vector.tensor_copy(v_sb[:, tg * 4:tg * 4 + 4, :], ps)

            # ---------------- phase 2+3: attention, collective, wo ----------
            with ExitStack() as p2:
                wst2 = p2.enter_context(tc.tile_pool(name="wstage2", bufs=1))
                wop = p2.enter_context(tc.tile_pool(name="woT", bufs=1))
                ptp = p2.enter_context(tc.tile_pool(name="pt", bufs=6))
                smp = p2.enter_context(tc.tile_pool(name="sm", bufs=3))
                otp = p2.enter_context(tc.tile_pool(name="ot", bufs=2))
                ovsp = p2.enter_context(tc.tile_pool(name="ovs", bufs=1))
                normp = p2.enter_context(tc.tile_pool(name="norm", bufs=2))
                gsb = p2.enter_context(tc.tile_pool(name="gsb", bufs=1))
                ostg = p2.enter_context(tc.tile_pool(name="ostage", bufs=3))
                maskp = p2.enter_context(tc.tile_pool(name="maskp", bufs=2))
                st_ps = p2.enter_context(
                    tc.tile_pool(name="st_ps", bufs=2, space="PSUM"))
                ov_ps = p2.enter_context(
                    tc.tile_pool(name="ov_ps", bufs=1, space="PSUM"))
                z_ps = p2.enter_context(
                    tc.tile_pool(name="z_ps", bufs=1, space="PSUM"))
                wo_ps = p2.enter_context(
                    tc.tile_pool(name="wo_ps", bufs=1, space="PSUM"))
                tp2_ps = p2.enter_context(
                    tc.tile_pool(name="tp2_ps", bufs=1, space="PSUM"))

                # wo^T (wq/wk/wv pools are closed now)
                woT = wop.tile([P, NDT, OQ], bf16)
                for pt in range(4):
                    stg = wst2.tile([P, D], bf16, tag="wstg2")
                    nc.gpsimd.dma_start(
                        out=stg, in_=wo_p[pt * P:(pt + 1) * P, :])
                    for dg in range(NDT // 4):
                        ps = tp2_ps.tile([P, 4, P], bf16, tag="tps2")
                        for j in range(4):
                            dt_i = dg * 4 + j
                            nc.tensor.transpose(
                                ps[:, j, :],
                                stg[:, dt_i * P:(dt_i + 1) * P], eye_bf)
                        nc.vector.tensor_copy(
                            woT[:, dg * 4:dg * 4 + 4, pt * P:(pt + 1) * P], ps)

                cc_ins = []
                gaths = []
                for qc in range(NCH):
                    cc_ins.append(dram.tile(
                        [OQ, SC], bf16, tag=f"ccin{qc}", name=f"ccin{qc}"))
                    gaths.append(dram.tile(
                        [NCORES * OQ, SC], bf16, tag=f"gath{qc}",
                        name=f"gath{qc}", addr_space="Shared"))

                for qc in range(NCH):
                    live, diag = k_tiles_for(qc)

                    # transposed additive mask for the tiles that need it
                    mt_tiles = {}
                    if diag:
                        dlist = sorted(diag)
                        mt = maskp.tile(
                            [P, len(dlist), SC], f32, tag="mt", bufs=1)
                        for g0 in range(0, len(dlist), 4):
                            grp = dlist[g0:g0 + 4]
                            mstg = maskp.tile(
                                [P, 4, len(grp) * P], f32, tag="mstg", bufs=1)
                            nc.sync.dma_start(
                                out=mstg,
                                in_=mask_p[qc * SC:(qc + 1) * SC,
                                           grp[0] * P:(grp[-1] + 1) * P]
                                .rearrange("(qs p) k -> p qs k", p=P))
                            for ji, ktile in enumerate(grp):
                                for qs in range(4):
                                    ps = tp2_ps.tile([P, P], f32, tag="mtps")
                                    nc.tensor.transpose(
                                        ps, mstg[:, qs, ji * P:(ji + 1) * P],
                                        eye_f)
                                    # clamp very negative mask values so exp
                                    # underflows cleanly
                                    nc.vector.tensor_scalar_max(
                                        mt[:, g0 + ji, qs * P:(qs + 1) * P],
                                        ps, NEG_CLAMP)
                                mt_tiles[ktile] = mt[:, g0 + ji, :]

                    ovs = ovsp.tile([P, H, SC], f32, tag="ovs")
                    zpack = normp.tile([1, H * SC], f32, tag="zpack", bufs=1)
                    for h in range(H):
                        ovp = ov_ps.tile([P, SC], f32, tag="ov")
                        zp = z_ps.tile([1, SC], f32, tag="z")
                        for ki, ktile in enumerate(live):
                            stp = st_ps.tile([P, SC], f32, tag="st")
                            nc.tensor.matmul(
                                stp, kt[:, ktile * P:(ktile + 1) * P],
                                qt[:, h, qc * SC:(qc + 1) * SC],
                                start=True, stop=True)
                            pt_t = ptp.tile([P, SC], bf16, tag="pt")
                            if ktile in mt_tiles:
                                sm = smp.tile([P, SC], f32, tag="sm")
                                nc.vector.scalar_tensor_tensor(
                                    sm, stp, INV_SQRT_DH, mt_tiles[ktile],
                                    MULT, ADD)
                                nc.scalar.activation(
                                    pt_t, sm, EXP, scale=1.0)
                            else:
                                nc.scalar.activation(
                                    pt_t, stp, EXP, scale=INV_SQRT_DH)
                            first = ki == 0
                            last = ki == len(live) - 1
                            nc.tensor.matmul(
                                ovp, v_sb[:, ktile, :], pt_t,
                                start=first, stop=last)
                            nc.tensor.matmul(
                                zp, ones_col, pt_t, start=first, stop=last)
                        nc.scalar.activation(ovs[:, h, :], ovp, COPY)
                        nc.scalar.activation(
                            zpack[:, h * SC:(h + 1) * SC], zp, COPY)

                    zrec = normp.tile([1, H * SC], f32, tag="zrec", bufs=1)
                    nc.vector.reciprocal(zrec, zpack)
                    ot = otp.tile([P, H, SC], bf16, tag="ot")
                    for h in range(H):
                        rec_sb = normp.tile([P, SC], f32, tag="recsb")
                        nc.gpsimd.partition_broadcast(
                            rec_sb, zrec[:, h * SC:(h + 1) * SC])
                        nc.vector.tensor_tensor(
                            ot[:, h, :], ovs[:, h, :], rec_sb, MULT)
                    nc.sync.dma_start(
                        out=cc_ins[qc].rearrange("(h p) q -> p h q", p=P),
                        in_=ot)
                    nc.gpsimd.collective_compute(
                        "AllGather",
                        mybir.AluOpType.bypass,
                        replica_groups=[list(range(NCORES))],
                        ins=[cc_ins[qc].opt()],
                        outs=[gaths[qc].opt()],
                    )

                    # wo for this q-chunk
                    g_t = gsb.tile([P, NDT, SC], bf16, tag="g")
                    nc.sync.dma_start(
                        out=g_t,
                        in_=gaths[qc].rearrange("(t p) q -> p t q", p=P))
                    for ss in range(4):
                        wps = wo_ps.tile([P, OQ], f32, tag="wo")
                        for d in range(NDT):
                            nc.tensor.matmul(
                                wps, g_t[:, d, ss * P:(ss + 1) * P],
                                woT[:, d, :],
                                start=(d == 0), stop=(d == NDT - 1))
                        o_t = ostg.tile([P, OQ], f32, tag="ostg")
                        nc.vector.tensor_copy(o_t, wps)
                        nc.sync.dma_start(
                            out=out_p[qc * SC + ss * P: qc * SC + (ss + 1) * P, :],
                            in_=o_t)

    nc.compile()
    return nc


def _get_nc(mask_mode):
    if mask_mode not in _BUILD_CACHE:
        _BUILD_CACHE[mask_mode] = _build(mask_mode)
    return _BUILD_CACHE[mask_mode]


def _mask_mode(mask):
    if not np.any(mask):
        return MODE_NONE
    kq = np.triu(np.full((S, S), -1e9, np.float32), k=1)
    if np.array_equal(mask, kq):
        return MODE_CAUSAL
    return MODE_GENERAL


def kernel(**inputs):
    x = np.ascontiguousarray(
        np.asarray(inputs["x"], dtype=np.float32).reshape(S, D))
    wq = np.asarray(inputs["wq"], dtype=np.float32)
    wk = np.asarray(inputs["wk"], dtype=np.float32)
    wv = np.asarray(inputs["wv"], dtype=np.float32)
    wo = np.asarray(inputs["wo"], dtype=np.float32)
    cosf = np.ascontiguousarray(np.asarray(inputs["freqs_cos"], np.float32))
    sinf = np.ascontiguousarray(np.asarray(inputs["freqs_sin"], np.float32))
    mask = np.asarray(inputs["mask"], dtype=np.float32)
    start_pos = int(np.asarray(inputs.get("start_pos", 0)))
    assert start_pos == 0, "kernel specialized for start_pos == 0"

    mode = _mask_mode(mask)
    nc = _get_nc(mode)

    in_maps = []
    for c in range(NCORES):
        m = {
            "x": x,
            "wq": np.ascontiguousarray(wq[c * OQ:(c + 1) * OQ]),
            "wk": np.ascontiguousarray(wk[c * DH:(c + 1) * DH]),
            "wv": np.ascontiguousarray(wv[c * DH:(c + 1) * DH]),
            "wo": np.ascontiguousarray(wo[c * OQ:(c + 1) * OQ]),
            "cosf": cosf,
            "sinf": sinf,
        }
        if mode != MODE_NONE:
            m["mask"] = np.ascontiguousarray(mask)
        in_maps.append(m)

    from concourse.bass_utils import run_bass_kernel_spmd

    res = run_bass_kernel_spmd(nc, in_maps, core_ids=list(range(NCORES)))
    outs = [r["out"] for r in res.results]
    full = np.concatenate(outs, axis=1).reshape(1, S, D)
    return np.ascontiguousarray(full.astype(np.float32))


# revision 42
# speedup vs baseline: 141.2537x; 141.2537x over previous
"""Distributed Trainium2 Bass kernel for nn_Attention_1726576855421.

Dense GQA attention block (dim 4096, 32 q-heads / 8 kv-heads, head_dim 128,
seq 2048, start_pos 0) tensor-parallel over heads across 8 NeuronCores:
  - core c owns q-heads [4c, 4c+4) and kv-head c
  - wq/wk/wv sharded on the output dim, wo sharded on the OUTPUT dim
    (each core computes a 512-wide column slice of the final output, so the
    host just concatenates along the feature axis; the only collective is a
    chunked AllGather of the per-core attention outputs).

Compute dtype: bf16 operands with fp32 PSUM accumulation (rel-err ~1e-3).
Scores are computed transposed (ST[k, q]) so that:
  - softmax denominator = ones-vector matmul over partitions,
  - P@V needs no transposition of P (lhsT = V natural, rhs = exp(ST)).
"""

import sys

for _p in ("/opt/trn_rl_repo", "/root/.axon_site/_ro/trn_rl_repo"):
    if _p not in sys.path:
        sys.path.append(_p)

import numpy as np

# problem constants (hardcoded per the task statement)
S = 2048          # sequence length
D = 4096          # model dim
NCORES = 8
H = 4             # q heads per core
DH = 128          # head dim
P = 128           # partitions
OQ = H * DH       # 512, per-core q-projection width
NDT = D // P      # 32 d-tiles
NKT = S // P      # 16 k-tiles
SC = 512          # s-chunk (free dim of most matmuls)
NCH = S // SC     # 4 chunks
NEG_CLAMP = -60.0
INV_SQRT_DH = float(1.0 / np.sqrt(DH))

MODE_NONE = "none"       # mask is all zeros -> no masking at all
MODE_CAUSAL = "causal"   # mask == triu(NEG_INF, k=1) -> skip masked tiles
MODE_GENERAL = "general" # arbitrary additive mask

_BUILD_CACHE = {}


def _build(mask_mode):
    import ml_dtypes
    import concourse.bacc as bacc
    import concourse.bass as bass
    import concourse.tile as tile
    import concourse.mybir as mybir

    f32 = mybir.dt.float32
    f32r = mybir.dt.float32r
    bf16 = mybir.dt.bfloat16
    EXP = mybir.ActivationFunctionType.Exp
    COPY = mybir.ActivationFunctionType.Copy
    MULT = mybir.AluOpType.mult
    ADD = mybir.AluOpType.add
    MAXOP = mybir.AluOpType.max
    npbf = ml_dtypes.bfloat16

    nc = bacc.Bacc(None, target_bir_lowering=False, debug=False)

    x_p = nc.declare_dram_parameter("x", [S, D], f32, isOutput=False)
    wq_p = nc.declare_dram_parameter("wq", [OQ, D], f32, isOutput=False)
    wk_p = nc.declare_dram_parameter("wk", [DH, D], f32, isOutput=False)
    wv_p = nc.declare_dram_parameter("wv", [DH, D], f32, isOutput=False)
    wo_p = nc.declare_dram_parameter("wo", [OQ, D], f32, isOutput=False)
    cos_p = nc.declare_dram_parameter("cosf", [S, DH // 2], f32, isOutput=False)
    sin_p = nc.declare_dram_parameter("sinf", [S, DH // 2], f32, isOutput=False)
    if mask_mode != MODE_NONE:
        mask_p = nc.declare_dram_parameter("mask", [S, S], f32, isOutput=False)
    out_p = nc.declare_dram_parameter("out", [S, OQ], f32, isOutput=True)

    # constants baked into the NEFF
    eye_bf_d = nc.inline_tensor(np.eye(P, dtype=npbf), name="eye_bf")
    eye_f_d = nc.inline_tensor(np.eye(P, dtype=np.float32), name="eye_f")
    rswap = np.zeros((P, P), npbf)
    for i in range(P):
        rswap[i ^ 1, i] = 1.0
    rswap_d = nc.inline_tensor(rswap, name="rswap")
    ones_col_d = nc.inline_tensor(np.ones((P, 1), dtype=npbf), name="ones_col")

    # which k-tiles are live / need the additive mask, per q-chunk
    def k_tiles_for(qc):
        if mask_mode == MODE_NONE:
            return list(range(NKT)), set()
        if mask_mode == MODE_GENERAL:
            return list(range(NKT)), set(range(NKT))
        # causal: k-tile fully unmasked iff kt*128+127 <= qc*512 (min q)
        live = list(range(4 * qc + 4))
        diag = set(range(4 * qc, 4 * qc + 4))
        return live, diag

    with tile.TileContext(nc) as tc:
        from contextlib import ExitStack

        with ExitStack() as top:
            consts = top.enter_context(tc.tile_pool(name="consts", bufs=1))
            dram = top.enter_context(tc.tile_pool(name="dram", bufs=1, space="DRAM"))

            eye_bf = consts.tile([P, P], bf16)
            nc.sync.dma_start(out=eye_bf, in_=eye_bf_d[:, :])
            eye_f = consts.tile([P, P], f32)
            nc.sync.dma_start(out=eye_f, in_=eye_f_d[:, :])
            rsw = consts.tile([P, P], bf16)
            nc.sync.dma_start(out=rsw, in_=rswap_d[:, :])
            ones_col = consts.tile([P, 1], bf16)
            nc.sync.dma_start(out=ones_col, in_=ones_col_d[:, :])

            # persistent activations
            qt = consts.tile([P, H, S], bf16)       # 2 MB, rope'd Q^T per head
            kt = consts.tile([P, S], bf16)          # 0.5 MB, rope'd K^T
            v_sb = consts.tile([P, NKT, DH], bf16)  # 0.5 MB, V natural

            # ---------------- phase 0c+1: weights + QKV ----------------
            with ExitStack() as p1:
                rope_consts = p1.enter_context(
                    tc.tile_pool(name="rope_consts", bufs=1))
                ct = rope_consts.tile([P, S], bf16)    # cos multiplier (transposed)
                st_m = rope_consts.tile([P, S], bf16)  # +-sin multiplier (transposed)

                def emit_rope_prep():
                    with tc.tile_pool(name="rope_prep", bufs=2) as rp, \
                         tc.tile_pool(name="rp_ps", bufs=2, space="PSUM") as rp_ps:
                        cos_sb = rp.tile([P, NKT, DH // 2], f32, tag="cs")
                        nc.sync.dma_start(
                            out=cos_sb,
                            in_=cos_p.ap().rearrange("(t p) f -> p t f", p=P)
                        )
                        sin_sb = rp.tile([P, NKT, DH // 2], f32, tag="cs")
                        nc.sync.dma_start(
                            out=sin_sb,
                            in_=sin_p.ap().rearrange("(t p) f -> p t f", p=P)
                        )
                        cexp = rp.tile([P, NKT, DH], bf16, tag="ce")
                        sexp = rp.tile([P, NKT, DH], bf16, tag="ce")
                        cview = cexp.rearrange("p t (f two) -> p t f two", two=2)
                        sview = sexp.rearrange("p t (f two) -> p t f two", two=2)
                        nc.vector.tensor_copy(cview[:, :, :, 0], cos_sb)
                        nc.vector.tensor_copy(cview[:, :, :, 1], cos_sb)
                        # S'[s, 2i] = -sin[s, i], S'[s, 2i+1] = +sin[s, i]
                        nc.vector.tensor_scalar_mul(
                            sview[:, :, :, 0], sin_sb, -1.0)
                        nc.vector.tensor_copy(sview[:, :, :, 1], sin_sb)
                        for t in range(NKT):
                            cps = rp_ps.tile([P, P], bf16, tag="cps")
                            nc.tensor.transpose(cps, cexp[:, t, :], eye_bf)
                            nc.vector.tensor_copy(ct[:, t * P:(t + 1) * P], cps)
                            sps = rp_ps.tile([P, P], bf16, tag="cps")
                            nc.tensor.transpose(sps, sexp[:, t, :], eye_bf)
                            nc.vector.tensor_copy(
                                st_m[:, t * P:(t + 1) * P], sps)

                wst = p1.enter_context(tc.tile_pool(name="wstage", bufs=3))
                wpool = p1.enter_context(tc.tile_pool(name="wqkvT", bufs=1))
                xtp = p1.enter_context(tc.tile_pool(name="xt", bufs=2))
                vtp = p1.enter_context(tc.tile_pool(name="vt", bufs=1))
                ropep = p1.enter_context(tc.tile_pool(name="ropep", bufs=2))
                tp_ps = p1.enter_context(
                    tc.tile_pool(name="tp_ps", bufs=2, space="PSUM"))
                qkv_ps = p1.enter_context(
                    tc.tile_pool(name="qkv_ps", bufs=2, space="PSUM"))
                rot_ps = p1.enter_context(
                    tc.tile_pool(name="rot_ps", bufs=2, space="PSUM"))

                wqT = wpool.tile([P, NDT, OQ], bf16)
                wkT = wpool.tile([P, NDT, DH], bf16)
                wvT = wpool.tile([P, NDT, DH], bf16)
                vt_sb = vtp.tile([P, S], bf16)

                # x cast to bf16 DRAM scratch (gpsimd cast-DMA), then xbar
                # DMA-transpose straight into SBUF — keeps the PE free.
                # Casts are emitted in 128-row slices interleaved with the
                # weight stage loads (same gpsimd queue) so neither starves.
                x_bf = dram.tile([S, D], bf16, tag="x_bf", name="x_bf")
                _xcast_next = [0]

                def emit_x_casts(n):
                    for _ in range(n):
                        st = _xcast_next[0]
                        if st >= NKT:
                            return
                        _xcast_next[0] += 1
                        nc.gpsimd.dma_start(
                            out=x_bf[st * P:(st + 1) * P, :],
                            in_=x_p[st * P:(st + 1) * P, :])

                def transpose_weight(w_param, n_pt, wT, evac_engines):
                    # w [n_pt*128, 4096] f32 DRAM -> wT [128, 32, n_pt*128] bf16
                    for pt in range(n_pt):
                        stg = wst.tile([P, D], bf16, tag="wstg")
                        nc.gpsimd.dma_start(
                            out=stg, in_=w_param[pt * P:(pt + 1) * P, :])
                        emit_x_casts(1)
                        for dg in range(NDT // 4):
                            ps = tp_ps.tile([P, 4, P], bf16, tag="tps")
                            for j in range(4):
                                dt_i = dg * 4 + j
                                nc.tensor.transpose(
                                    ps[:, j, :],
                                    stg[:, dt_i * P:(dt_i + 1) * P], eye_bf)
                            eng = evac_engines[dg % len(evac_engines)]
                            eng.tensor_copy(
                                wT[:, dg * 4:dg * 4 + 4, pt * P:(pt + 1) * P], ps)

                transpose_weight(wq_p, 4, wqT, [nc.vector])
                emit_rope_prep()
                transpose_weight(wk_p, 1, wkT, [nc.vector])
                transpose_weight(wv_p, 1, wvT, [nc.vector])
                emit_x_casts(NKT)

                def rope_evac(psum, rot_src, dst, c):
                    # dst[:, c*SC:(c+1)*SC] = psum*ct + (R@psum)*st  (all rope'd)
                    raw = ropep.tile([P, SC], bf16, tag="raw")
                    nc.scalar.activation(raw, psum, COPY)
                    rps = rot_ps.tile([P, SC], f32, tag="rot")
                    nc.tensor.matmul(rps, rsw, raw, start=True, stop=True)
                    rotb = ropep.tile([P, SC], bf16, tag="rotb")
                    nc.vector.tensor_copy(rotb, rps)
                    t1 = ropep.tile([P, SC], bf16, tag="t1")
                    nc.vector.tensor_tensor(
                        t1, raw, ct[:, c * SC:(c + 1) * SC], MULT)
                    t2 = ropep.tile([P, SC], bf16, tag="t2")
                    nc.vector.tensor_tensor(
                        t2, rotb, st_m[:, c * SC:(c + 1) * SC], MULT)
                    nc.vector.tensor_tensor(dst, t1, t2, ADD)

                for c in range(NCH):
                    xt_c = xtp.tile([P, NDT, SC], bf16, tag="xt")
                    nc.sync.dma_start_transpose(
                        xt_c, x_bf[c * SC:(c + 1) * SC, :])
                    # Q^T per head
                    for h in range(H):
                        ps = qkv_ps.tile([P, SC], f32, tag="qkv")
                        for d in range(NDT):
                            nc.tensor.matmul(
                                ps, wqT[:, d, h * P:(h + 1) * P], xt_c[:, d, :],
                                start=(d == 0), stop=(d == NDT - 1))
                        rope_evac(ps, rot_ps, qt[:, h, c * SC:(c + 1) * SC], c)
                    # K^T
                    ps = qkv_ps.tile([P, SC], f32, tag="qkv")
                    for d in range(NDT):
                        nc.tensor.matmul(
                            ps, wkT[:, d, :], xt_c[:, d, :],
                            start=(d == 0), stop=(d == NDT - 1))
                    rope_evac(ps, rot_ps, kt[:, c * SC:(c + 1) * SC], c)
                    # V^T (no rope)
                    ps = qkv_ps.tile([P, SC], f32, tag="qkv")
                    for d in range(NDT):
                        nc.tensor.matmul(
                            ps, wvT[:, d, :], xt_c[:, d, :],
                            start=(d == 0), stop=(d == NDT - 1))
                    nc.scalar.activation(
                        vt_sb[:, c * SC:(c + 1) * SC], ps, COPY)

                # V natural [s, d] from V^T
                for tg in range(NKT // 4):
                    ps = tp_ps.tile([P, 4, P], bf16, tag="tps")
                    for j in range(4):
                        t = tg * 4 + j
                        nc.tensor.transpose(
                            ps[:, j, :], vt_sb[:, t * P:(t + 1) * P], eye_bf)
                    nc.vector.tensor_copy(v_sb[:, tg * 4:tg * 4 + 4, :], ps)

            # ---------------- phase 2+3: attention, collective, wo ----------
            with ExitStack() as p2:
                wst2 = p2.enter_context(tc.tile_pool(name="wstage2", bufs=1))
                wop = p2.enter_context(tc.tile_pool(name="woT", bufs=1))
                ptp = p2.enter_context(tc.tile_pool(name="pt", bufs=6))
                smp = p2.enter_context(tc.tile_pool(name="sm", bufs=3))
                otp = p2.enter_context(tc.tile_pool(name="ot", bufs=2))
                ovsp = p2.enter_context(tc.tile_pool(name="ovs", bufs=1))
                normp = p2.enter_context(tc.tile_pool(name="norm", bufs=2))
                gsb = p2.enter_context(tc.tile_pool(name="gsb", bufs=2))
                ostg = p2.enter_context(tc.tile_pool(name="ostage", bufs=3))
                maskp = p2.enter_context(tc.tile_pool(name="maskp", bufs=2))
                st_ps = p2.enter_context(
                    tc.tile_pool(name="st_ps", bufs=3, space="PSUM"))
                ov_ps = p2.enter_context(
                    tc.tile_pool(name="ov_ps", bufs=1, space="PSUM"))
                z_ps = p2.enter_context(
                    tc.tile_pool(name="z_ps", bufs=1, space="PSUM"))
                wo_ps = p2.enter_context(
                    tc.tile_pool(name="wo_ps", bufs=1, space="PSUM"))
                tp2_ps = p2.enter_context(
                    tc.tile_pool(name="tp2_ps", bufs=1, space="PSUM"))

                # wo^T (wq/wk/wv pools are closed now)
                woT = wop.tile([P, NDT, OQ], bf16)
                for pt in range(4):
                    stg = wst2.tile([P, D], bf16, tag="wstg2")
                    nc.gpsimd.dma_start(
                        out=stg, in_=wo_p[pt * P:(pt + 1) * P, :])
                    for dg in range(NDT // 4):
                        ps = tp2_ps.tile([P, 4, P], bf16, tag="tps2")
                        for j in range(4):
                            dt_i = dg * 4 + j
                            nc.tensor.transpose(
                                ps[:, j, :],
                                stg[:, dt_i * P:(dt_i + 1) * P], eye_bf)
                        nc.vector.tensor_copy(
                            woT[:, dg * 4:dg * 4 + 4, pt * P:(pt + 1) * P], ps)

                cc_ins = []
                gaths = []
                for qc in range(NCH):
                    cc_ins.append(dram.tile(
                        [OQ, SC], bf16, tag=f"ccin{qc}", name=f"ccin{qc}"))
                    gaths.append(dram.tile(
                        [NCORES * OQ, SC], bf16, tag=f"gath{qc}",
                        name=f"gath{qc}", addr_space="Shared"))

                def attention_chunk(qc):
                    live, diag = k_tiles_for(qc)

                    # transposed additive mask for the tiles that need it
                    mt_tiles = {}
                    if diag:
                        dlist = sorted(diag)
                        mt = maskp.tile(
                            [P, len(dlist), SC], f32, tag="mt", bufs=1)
                        for g0 in range(0, len(dlist), 4):
                            grp = dlist[g0:g0 + 4]
                            mstg = maskp.tile(
                                [P, 4, len(grp) * P], f32, tag="mstg", bufs=1)
                            nc.sync.dma_start(
                                out=mstg,
                                in_=mask_p[qc * SC:(qc + 1) * SC,
                                           grp[0] * P:(grp[-1] + 1) * P]
                                .rearrange("(qs p) k -> p qs k", p=P))
                            for ji, ktile in enumerate(grp):
                                for qs in range(4):
                                    ps = tp2_ps.tile([P, P], f32, tag="mtps")
                                    nc.tensor.transpose(
                                        ps, mstg[:, qs, ji * P:(ji + 1) * P],
                                        eye_f)
                                    # clamp very negative mask values so exp
                                    # underflows cleanly
                                    nc.vector.tensor_scalar_max(
                                        mt[:, g0 + ji, qs * P:(qs + 1) * P],
                                        ps, NEG_CLAMP)
                                mt_tiles[ktile] = mt[:, g0 + ji, :]

                    ovs = ovsp.tile([P, H, SC], f32, tag="ovs")
                    zpack = normp.tile([1, H * SC], f32, tag="zpack", bufs=1)
                    for h in range(H):
                        ovp = ov_ps.tile([P, SC], f32, tag="ov")
                        zp = z_ps.tile([1, SC], f32, tag="z")
                        n_live = len(live)

                        # two-deep software pipeline: issue ST(k+1), ST(k+2)
                        # before AV(k)/Z(k) so the PE never waits on the exp.
                        pending = []

                        def flush_one():
                            ki, ktile, pt_t = pending.pop(0)
                            first = ki == 0
                            last = ki == n_live - 1
                            nc.tensor.matmul(
                                ovp, v_sb[:, ktile, :], pt_t,
                                start=first, stop=last)
                            nc.tensor.matmul(
                                zp, ones_col, pt_t, start=first, stop=last)

                        for ki, ktile in enumerate(live):
                            stp = st_ps.tile([P, SC], f32, tag="st")
                            nc.tensor.matmul(
                                stp, kt[:, ktile * P:(ktile + 1) * P],
                                qt[:, h, qc * SC:(qc + 1) * SC],
                                start=True, stop=True)
                            pt_t = ptp.tile([P, SC], bf16, tag="pt")
                            if ktile in mt_tiles:
                                sm = smp.tile([P, SC], f32, tag="sm")
                                nc.vector.scalar_tensor_tensor(
                                    sm, stp, INV_SQRT_DH, mt_tiles[ktile],
                                    MULT, ADD)
                                nc.scalar.activation(
                                    pt_t, sm, EXP, scale=1.0)
                            else:
                                nc.scalar.activation(
                                    pt_t, stp, EXP, scale=INV_SQRT_DH)
                            pending.append((ki, ktile, pt_t))
                            if len(pending) > 2:
                                flush_one()
                        while pending:
                            flush_one()
                        nc.vector.tensor_copy(ovs[:, h, :], ovp)
                        nc.scalar.activation(
                            zpack[:, h * SC:(h + 1) * SC], zp, COPY)

                    zrec = normp.tile([1, H * SC], f32, tag="zrec", bufs=1)
                    nc.vector.reciprocal(zrec, zpack)
                    ot = otp.tile([P, H, SC], bf16, tag="ot")
                    for h in range(H):
                        rec_sb = normp.tile([P, SC], f32, tag="recsb")
                        nc.gpsimd.partition_broadcast(
                            rec_sb, zrec[:, h * SC:(h + 1) * SC])
                        nc.vector.tensor_tensor(
                            ot[:, h, :], ovs[:, h, :], rec_sb, MULT)
                    nc.sync.dma_start(
                        out=cc_ins[qc].rearrange("(h p) q -> p h q", p=P),
                        in_=ot)
                    nc.gpsimd.collective_compute(
                        "AllGather",
                        mybir.AluOpType.bypass,
                        replica_groups=[list(range(NCORES))],
                        ins=[cc_ins[qc].opt()],
                        outs=[gaths[qc].opt()],
                    )

                def wo_chunk(qc):
                    g_t = gsb.tile([P, NDT, SC], bf16, tag="g")
                    nc.sync.dma_start(
                        out=g_t,
                        in_=gaths[qc].rearrange("(t p) q -> p t q", p=P))
                    for ss in range(4):
                        wps = wo_ps.tile([P, OQ], f32, tag="wo")
                        for d in range(NDT):
                            nc.tensor.matmul(
                                wps, g_t[:, d, ss * P:(ss + 1) * P],
                                woT[:, d, :],
                                start=(d == 0), stop=(d == NDT - 1))
                        o_t = ostg.tile([P, OQ], f32, tag="ostg")
                        nc.vector.tensor_copy(o_t, wps)
                        nc.sync.dma_start(
                            out=out_p[qc * SC + ss * P: qc * SC + (ss + 1) * P, :],
                            in_=o_t)

                # software pipeline: wo(qc-1) is emitted after attention(qc),
                # so the PE never head-of-line blocks on the AllGather of qc-1.
                for qc in range(NCH):
                    attention_chunk(qc)
                    if qc > 0:
                        wo_chunk(qc - 1)
                wo_chunk(NCH - 1)

    nc.compile()
    return nc


def _get_nc(mask_mode):
    if mask_mode not in _BUILD_CACHE:
        _BUILD_CACHE[mask_mode] = _build(mask_mode)
    return _BUILD_CACHE[mask_mode]


def _mask_mode(mask):
    if not np.any(mask):
        return MODE_NONE
    kq = np.triu(np.full((S, S), -1e9, np.float32), k=1)
    if np.array_equal(mask, kq):
        return MODE_CAUSAL
    return MODE_GENERAL


def kernel(**inputs):
    x = np.ascontiguousarray(
        np.asarray(inputs["x"], dtype=np.float32).reshape(S, D))
    wq = np.asarray(inputs["wq"], dtype=np.float32)
    wk = np.asarray(inputs["wk"], dtype=np.float32)
    wv = np.asarray(inputs["wv"], dtype=np.float32)
    wo = np.asarray(inputs["wo"], dtype=np.float32)
    cosf = np.ascontiguousarray(np.asarray(inputs["freqs_cos"], np.float32))
    sinf = np.ascontiguousarray(np.asarray(inputs["freqs_sin"], np.float32))
    mask = np.asarray(inputs["mask"], dtype=np.float32)
    start_pos = int(np.asarray(inputs.get("start_pos", 0)))
    assert start_pos == 0, "kernel specialized for start_pos == 0"

    mode = _mask_mode(mask)
    nc = _get_nc(mode)

    in_maps = []
    for c in range(NCORES):
        m = {
            "x": x,
            "wq": np.ascontiguousarray(wq[c * OQ:(c + 1) * OQ]),
            "wk": np.ascontiguousarray(wk[c * DH:(c + 1) * DH]),
            "wv": np.ascontiguousarray(wv[c * DH:(c + 1) * DH]),
            "wo": np.ascontiguousarray(wo[c * OQ:(c + 1) * OQ]),
            "cosf": cosf,
            "sinf": sinf,
        }
        if mode != MODE_NONE:
            m["mask"] = np.ascontiguousarray(mask)
        in_maps.append(m)

    from concourse.bass_utils import run_bass_kernel_spmd

    res = run_bass_kernel_spmd(nc, in_maps, core_ids=list(range(NCORES)))
    outs = [r["out"] for r in res.results]
    full = np.concatenate(outs, axis=1).reshape(1, S, D)
    return np.ascontiguousarray(full.astype(np.float32))
